# revision 21
# baseline (speedup 1.0000x reference)
"""Trainium2 Bass kernel for CausalSelfAttention (B=8, N=1024, C=768, H=12).

Sharding: data-parallel over batch - one batch element per NeuronCore,
weights replicated, no collectives.

v2 design (vs the 223us baseline):
  - x is transposed and fp8-pair-packed on the HOST (free), killing all
    on-chip PE transposes and their DVE copybacks.
  - fp8 budget: only es (exp output) and the packed v tiles are fp8 -
    each fp8 tensor feeding a zero-mean contraction costs ~1.2% final
    rel error (no averaging benefit), so the stack must stay under the
    2e-2 gate: es+v fp8 = ~1.7%, adding qk fp8 would tip it to ~2.2%.
    q/k, v-compute, and projection matmuls stay fp16.
  - score matmuls (K=64) run two heads concurrently in the PE array via
    tile_position row packing (rows 0-63 / 64-127).
  - exp runs on ACT straight to fp8 es tiles laid out as DoubleRow kt
    pairs; PV runs fp8 DoubleRow over 256-token contractions with the
    rowsum ones-column folded into the packed v tiles.
  - bias algebra: k-bias cancels in softmax (terms depend only on q);
    v-bias is folded into an effective b_proj on the host; only the
    q-bias survives on-chip (6 DVE tensor-scalar adds).
  - normalization: reciprocal of the PV rowsum row, DRAM-bounce
    partition-broadcast, one fp16 multiply per chunk, trailing the
    pipeline by a head pair.
  - output projection split: contraction chunks 0-4 run inside the
    attention phase once heads 0-9 are normalized; only the last chunk
    (heads 10/11) plus a DVE add remains as the serial tail.
"""

import sys
import types

import numpy as np
import ml_dtypes

import concourse.bass as bass
import concourse.tile as tile
from concourse import bacc
from concourse import mybir

F32 = mybir.dt.float32
F16 = mybir.dt.float16
F8 = mybir.dt.float8e4
AF = mybir.ActivationFunctionType
DR = mybir.MatmulPerfMode.DoubleRow

B, N, C, H, D = 8, 1024, 768, 12, 64
CK = C // 128        # 6 fp16 contraction chunks
JK = C // 256        # 3 DoubleRow contraction chunks
NT = N // 128        # 8 token tiles
QC = N // 512        # 2 moving chunks of 512 tokens
NP = NT // 2         # 4 kt pair tiles for DoubleRow PV
VW = 80              # padded per-head width in packed v tiles (stride %16)
SCALE = 1.0 / np.sqrt(D)
USE_DR_PV = True


def _install_ntff_hook():
    """Register the axon NTFF profiling hook if the image's antenv lacks it."""
    try:
        from antenv.axon_hooks import get_axon_ntff_profile_hook  # noqa: F401
        return
    except ImportError:
        pass
    try:
        import antenv
        mod = types.ModuleType("antenv.axon_hooks")
        _h = [None]
        mod.set_axon_ntff_profile_hook = lambda h: _h.__setitem__(0, h)
        mod.get_axon_ntff_profile_hook = lambda: _h[0]
        antenv.axon_hooks = mod
        sys.modules["antenv.axon_hooks"] = mod
        if "/root/.axon_site" not in sys.path:
            sys.path.insert(0, "/root/.axon_site")
        from trn_agent_boot.trn_boot import _ntff_profile_via_ctypes
        hook = _ntff_profile_via_ctypes("/opt/axon/libaxon_pjrt.so")
        if hook is not None:
            mod.set_axon_ntff_profile_hook(hook)
    except Exception:
        pass


def build_bass(debug=False):
    nc = bacc.Bacc("TRN2", target_bir_lowering=False, debug=False)
    xT = nc.dram_tensor("xT", [CK, 128, N], F16, kind="ExternalInput").ap()
    waq = nc.dram_tensor("waq", [CK, 128, 2 * C], F16, kind="ExternalInput").ap()
    wav = nc.dram_tensor("wav", [CK, 128, C], F16, kind="ExternalInput").ap()
    wp = nc.dram_tensor("wp", [CK, 128, C], F16, kind="ExternalInput").ap()
    bq = nc.dram_tensor("bq", [128, CK], F32, kind="ExternalInput").ap()
    bp = nc.dram_tensor("bp", [1, C], F16, kind="ExternalInput").ap()
    y = nc.dram_tensor("y", [N, C], F16, kind="ExternalOutput").ap()
    dbg = None
    if debug:
        dbg = {
            "dbg_qT": nc.dram_tensor("dbg_qT", [CK, 128, N], F16, kind="ExternalOutput").ap(),
            "dbg_kT": nc.dram_tensor("dbg_kT", [CK, 128, N], F16, kind="ExternalOutput").ap(),
            "dbg_ao": nc.dram_tensor("dbg_ao", [CK, 128, N], F16, kind="ExternalOutput").ap(),
            "dbg_vp": nc.dram_tensor("dbg_vp", [NP, 128, 2, H, D + 1], F8, kind="ExternalOutput").ap(),
            "dbg_esa": nc.dram_tensor("dbg_esa", [128, NT, N], F8, kind="ExternalOutput").ap(),
            "dbg_esb": nc.dram_tensor("dbg_esb", [128, NT, N], F8, kind="ExternalOutput").ap(),
            "dbg_pvc": nc.dram_tensor("dbg_pvc", [QC, D + 1, 512], F32, kind="ExternalOutput").ap(),
            "dbg_rsi": nc.dram_tensor("dbg_rsi", [QC, 1, 512], F32, kind="ExternalOutput").ap(),
            "dbg_rbc": nc.dram_tensor("dbg_rbc", [QC, D, 512], F32, kind="ExternalOutput").ap(),
        }

    with tile.TileContext(nc) as tc:
        build_body(nc, tc, xT, waq, wav, wp, bq, bp, y, dbg)
    nc.compile()
    return nc


def build_body(nc, tc, xT_d, waq_d, wav_d, wp_d, bq_d, bp_d, y_d, dbg=None):
    from contextlib import ExitStack

    ctx = ExitStack()
    with ctx:
        singles = ctx.enter_context(tc.tile_pool(name="singles", bufs=1))
        persist = ctx.enter_context(tc.tile_pool(name="persist", bufs=1))
        p_e = ctx.enter_context(tc.tile_pool(name="exps", bufs=3))
        p_n = ctx.enter_context(tc.tile_pool(name="norm", bufs=4))
        p_ys = ctx.enter_context(tc.tile_pool(name="ysb", bufs=2))
        # PSUM budget (8 banks): scores 2 tags x 1 buf x 2 banks = 4
        # (A/B tag alternation keeps ACT saturated without double-buffering),
        # pv 2 bufs x 1 bank = 2, filler "mm" [128,1024] 1 buf x 2 banks = 2.
        p_sc = ctx.enter_context(tc.tile_pool(name="scpsum", bufs=1, space="PSUM"))
        p_pv = ctx.enter_context(tc.tile_pool(name="pvpsum", bufs=2, space="PSUM"))
        p_mm = ctx.enter_context(tc.tile_pool(name="mmpsum", bufs=1, space="PSUM"))
        p_dr = ctx.enter_context(tc.tile_pool(name="drscratch", bufs=4, space="DRAM"))

        ones = singles.tile([1, 128], F16, tag="ones")
        nc.gpsimd.memset(ones[:], 1.0)
        b_q = singles.tile([128, CK], F32, tag="b_q")
        nc.sync.dma_start(out=b_q[:], in_=bq_d[:, :])
        bp_row = singles.tile([1, C], F16, tag="bp_row")
        nc.gpsimd.dma_start(out=bp_row[:], in_=bp_d[:, :])

        # persistent SBUF tensors
        xT = [persist.tile([128, N], F16, name=f"xT{i}", tag=f"xT{i}") for i in range(CK)]
        waq = [persist.tile([128, 2 * C], F16, name=f"waq{i}", tag=f"waq{i}") for i in range(CK)]
        wav = [persist.tile([128, C], F16, name=f"wav{i}", tag=f"wav{i}") for i in range(CK)]
        wps = [persist.tile([128, C], F16, name=f"wp{i}", tag=f"wp{i}") for i in range(CK)]
        qT = [persist.tile([128, N], F16, name=f"qT{i}", tag=f"qT{i}") for i in range(CK)]
        kT = [persist.tile([128, N], F16, name=f"kT{i}", tag=f"kT{i}") for i in range(CK)]
        vpk = [persist.tile([128, 2, H, VW], F8, name=f"vp{j}", tag=f"vp{j}") for j in range(NP)]
        aout = [persist.tile([128, N], F16, name=f"ao{i}", tag=f"ao{i}") for i in range(CK)]
        ya = [persist.tile([128, C], F16, name=f"ya{t}", tag=f"ya{t}") for t in range(NT)]

        # input DMAs (front-load the ones the ramp needs)
        for i in range(CK):
            nc.sync.dma_start(out=xT[i][:], in_=xT_d[i])
            nc.sync.dma_start(out=waq[i][:], in_=waq_d[i])
        for i in range(CK):
            nc.scalar.dma_start(out=wav[i][:], in_=wav_d[i])
        for i in range(CK):
            nc.gpsimd.dma_start(out=wps[i][:], in_=wp_d[i])
        for j in range(NP):
            nc.gpsimd.memset(vpk[j][:, :, :, D:D + 1], 1.0)

        # ~2.5us of tiny matmuls to warm the PE HAM while input DMAs land
        ident = singles.tile([128, 128], F16, tag="ident")
        nc.gpsimd.memset(ident[:], 0.0)
        for _ in range(30):
            wp_ps = p_mm.tile([128, 1024], F32, name="warm", tag="mm")
            nc.tensor.matmul(wp_ps[:, 0:128], ident[:], ident[:], start=True, stop=True)

        def emit_qk(m):
            """q^T (m<6) or k^T (m>=6) chunk m: [128, 1024] fp16."""
            p = p_mm.tile([128, 1024], F32, name="qkp", tag="mm")
            for qc in range(QC):
                for ci in range(CK):
                    nc.tensor.matmul(
                        p[:, qc * 512:(qc + 1) * 512],
                        waq[ci][:, m * 128:(m + 1) * 128],
                        xT[ci][:, qc * 512:(qc + 1) * 512],
                        start=(ci == 0),
                        stop=(ci == CK - 1),
                    )
            if m < CK:
                nc.vector.tensor_scalar_add(qT[m][:], p[:], b_q[:, m:m + 1])
            else:
                nc.vector.tensor_copy(out=kT[m - CK][:], in_=p[:])

        def emit_v_tile(t):
            """v rows for token tile t -> packed fp8 pair tile (fp16 matmul)."""
            p = p_mm.tile([128, 1024], F32, name="vp", tag="mm")
            for off, w in ((0, 512), (512, 256)):
                for ci in range(CK):
                    nc.tensor.matmul(
                        p[:, off:off + w],
                        xT[ci][:, t * 128:(t + 1) * 128],
                        wav[ci][:, off:off + w],
                        start=(ci == 0),
                        stop=(ci == CK - 1),
                    )
            nc.vector.tensor_copy(
                out=vpk[t // 2][:, t % 2, :, 0:D],
                in_=p[:, 0:C].rearrange("p (h d) -> p h d", d=D),
            )

        def emit_scores_kt(pair, kt, es_a, es_b):
            """Row-packed scores for heads (2*pair, 2*pair+1) at key tile kt."""
            hq, hk = qT[pair], kT[pair]
            for half, es in ((0, es_a), (1, es_b)):
                ps = p_sc.tile([128, 1024], F32, name=f"sc{half}", tag=f"sc{half}")
                lo = half * D
                for qc in range(QC):
                    nc.tensor.matmul(
                        ps[:, qc * 512:(qc + 1) * 512],
                        hk[lo:lo + D, kt * 128:(kt + 1) * 128],
                        hq[lo:lo + D, qc * 512:(qc + 1) * 512],
                        start=True,
                        stop=True,
                        tile_position=(lo, 0),
                    )
                nc.scalar.activation(
                    out=es[:, kt, :], in_=ps[:], func=AF.Exp, scale=float(SCALE)
                )

        def emit_pv_j(h, es, pvs, j):
            """Accumulate PV pair j for head h into pvs[qc]."""
            for qc in range(QC):
                if USE_DR_PV:
                    nc.tensor.matmul(
                        pvs[qc][:],
                        vpk[j][:, :, h, 0:D + 1],
                        es[:, 2 * j:2 * j + 2, qc * 512:(qc + 1) * 512],
                        start=(j == 0),
                        stop=(j == NP - 1),
                        perf_mode=DR,
                    )
                else:
                    for i in range(2):
                        nc.tensor.matmul(
                            pvs[qc][:],
                            vpk[j][:, i, h, 0:D + 1],
                            es[:, 2 * j + i, qc * 512:(qc + 1) * 512],
                            start=(j == 0 and i == 0),
                            stop=(j == NP - 1 and i == 1),
                        )

        def emit_pv_drain(h, pvs):
            """Copy PV out of PSUM, start the rowsum-reciprocal DRAM bounce."""
            out = []
            for qc in range(QC):
                pv = pvs[qc]
                # reciprocal_approx_fast is a bitwise custom DVE op: on
                # silicon it can neither read PSUM nor shift base partitions,
                # so the raw rowsum row bounces through DRAM (partition 64 ->
                # broadcast on 0..63) and the reciprocal runs aligned later.
                pvc = p_n.tile([D + 1, 512], F32, name="pvc", tag="pvc", bufs=6)
                nc.vector.tensor_copy(out=pvc[:], in_=pv[0:D + 1, :])
                rs_d = p_dr.tile([1, 512], F32, name="rs_d", tag="rs_d")
                nc.sync.dma_start(out=rs_d[:], in_=pvc[D:D + 1, :])
                rbc_raw = p_n.tile([D, 512], F32, name="rbcr", tag="rbcr", bufs=4)
                nc.sync.dma_start(out=rbc_raw[:], in_=rs_d[0, :].partition_broadcast(D))
                out.append((pvc, rbc_raw))
            return out

        def emit_norm_mul(h, drained):
            for qc in range(QC):
                pvc, rbc_raw = drained[qc]
                rbc = p_n.tile([D, 512], F32, name="rbc", tag="rbc", bufs=4)
                nc.vector.reciprocal_approx_fast(out=rbc[:], in_=rbc_raw[:])
                if dbg is not None and h == H - 1:
                    nc.sync.dma_start(out=dbg["dbg_pvc"][qc], in_=pvc[0:D + 1, :])
                    nc.sync.dma_start(out=dbg["dbg_rbc"][qc], in_=rbc[:])
                nc.vector.tensor_mul(
                    aout[h // 2][(h % 2) * D:(h % 2) * D + D,
                                 qc * 512:(qc + 1) * 512],
                    pvc[0:D, :],
                    rbc[:],
                )

        def emit_proj_a(t):
            """Output projection over contraction chunks 0-4 (+bias) for tile t."""
            p = p_mm.tile([128, 1024], F32, name="ypa", tag="mm")
            for off, w in ((0, 512), (512, 256)):
                for ci in range(CK - 1):
                    nc.tensor.matmul(
                        p[:, off:off + w],
                        aout[ci][:, t * 128:(t + 1) * 128],
                        wps[ci][:, off:off + w],
                        start=(ci == 0),
                        stop=False,
                    )
                nc.tensor.matmul(
                    p[:, off:off + w],
                    ones[0:1, 0:128],
                    bp_row[0:1, off:off + w],
                    start=False,
                    stop=True,
                )
            nc.vector.tensor_copy(out=ya[t][:], in_=p[:, 0:C])

        def emit_proj_b(t):
            """Last contraction chunk (heads 10/11), add to ya, store y."""
            p = p_mm.tile([128, 1024], F32, name="ypb", tag="mm")
            for off, w in ((0, 512), (512, 256)):
                nc.tensor.matmul(
                    p[:, off:off + w],
                    aout[CK - 1][:, t * 128:(t + 1) * 128],
                    wps[CK - 1][:, off:off + w],
                    start=True,
                    stop=True,
                )
            ysb = p_ys.tile([128, C], F16, tag="ysb")
            nc.vector.tensor_add(ysb[:], ya[t][:], p[:, 0:C])
            nc.sync.dma_start(out=y_d[t * 128:(t + 1) * 128, :], in_=ysb[:])

        # ---------------- pipeline ----------------
        # filler: closures the PE chews on while ACT streams exps
        filler = [(lambda m=m: emit_qk(m)) for m in (0, CK)]
        filler += [(lambda t=t: emit_v_tile(t)) for t in (0, 1)]
        for q in filler:
            q()

        filler = []
        filler += [(lambda t=t: emit_v_tile(t)) for t in range(2, NT)]
        for i in range(1, H // 2):
            filler.append(lambda m=i: emit_qk(m))
            filler.append(lambda m=CK + i: emit_qk(m))

        pending_mul = []   # (head, drained) waiting a safe distance
        pending_pv_b = []  # head B pv closures, run early next iter

        def filler_step(k=1):
            for _ in range(k):
                if pending_mul:
                    h, dr = pending_mul.pop(0)
                    emit_norm_mul(h, dr)
                    continue
                if filler:
                    filler.pop(0)()

        for pair in range(H // 2):
            hA, hB = 2 * pair, 2 * pair + 1
            es_a = p_e.tile([128, NT, N], F8, name="esa", tag="es")
            es_b = p_e.tile([128, NT, N], F8, name="esb", tag="es")
            pvsA = None
            for kt in range(NT):
                emit_scores_kt(pair, kt, es_a, es_b)
                if kt == 0 and pending_pv_b:
                    # previous pair's head B: es complete, banks now free
                    pending_pv_b.pop(0)()
                if kt % 2 == 1:
                    j = kt // 2
                    if j == 0:
                        pvsA = [p_pv.tile([D + 1, 512], F32, name=f"pva{qc}", tag="pv")
                                for qc in range(QC)]
                    emit_pv_j(hA, es_a, pvsA, j)
                    filler_step(1)
                else:
                    filler_step(1)
            pending_mul.append((hA, emit_pv_drain(hA, pvsA)))

            def run_pv_b(h=hB, es=es_b):
                pvs = [p_pv.tile([D + 1, 512], F32, name=f"pvb{qc}", tag="pv")
                       for qc in range(QC)]
                for j in range(NP):
                    emit_pv_j(h, es, pvs, j)
                pending_mul.append((h, emit_pv_drain(h, pvs)))
            pending_pv_b.append(run_pv_b)

            if pair == H // 2 - 1:
                pending_pv_b.pop(0)()
            if pair == H // 2 - 2:
                # after this pair's muls run, heads 0-9 will be complete;
                # queue proj chunk A behind them
                filler += [(lambda t=t: emit_proj_a(t)) for t in range(NT)]

        # drain the tail: remaining muls (heads 10, 11), proj A remainder, proj B
        while pending_mul or filler:
            filler_step(1)
        for t in range(NT):
            emit_proj_b(t)

        if dbg is not None:
            for i in range(CK):
                nc.sync.dma_start(out=dbg["dbg_qT"][i], in_=qT[i][:])
                nc.sync.dma_start(out=dbg["dbg_kT"][i], in_=kT[i][:])
                nc.sync.dma_start(out=dbg["dbg_ao"][i], in_=aout[i][:])
            for j in range(NP):
                nc.sync.dma_start(out=dbg["dbg_vp"][j], in_=vpk[j][:, :, :, 0:D + 1])
            nc.sync.dma_start(out=dbg["dbg_esa"], in_=es_a[:])
            nc.sync.dma_start(out=dbg["dbg_esb"], in_=es_b[:])


_CACHE = {}


def kernel(x, pad_mask=None, w_attn=None, b_attn=None, w_proj=None, b_proj=None,
           _trace=False, _tmpdir=None):
    from concourse.bass_utils import run_bass_kernel_spmd

    FP8 = ml_dtypes.float8_e4m3

    x = np.asarray(x, dtype=np.float32)
    w_attn = np.asarray(w_attn, dtype=np.float32)
    b_attn = np.asarray(b_attn, dtype=np.float32)
    w_proj = np.asarray(w_proj, dtype=np.float32)
    b_proj = np.asarray(b_proj, dtype=np.float32)

    # host-side packing (free): transpose x
    xT = np.ascontiguousarray(
        x.transpose(0, 2, 1).reshape(B, CK, 128, N).astype(np.float16))
    waq16 = np.ascontiguousarray(
        w_attn[:, :2 * C].reshape(CK, 128, 2 * C)).astype(np.float16)
    wav16 = np.ascontiguousarray(
        w_attn[:, 2 * C:].reshape(CK, 128, C)).astype(np.float16)
    wp16 = np.ascontiguousarray(w_proj.reshape(CK, 128, C)).astype(np.float16)
    bq = np.ascontiguousarray(b_attn[:C].reshape(CK, 128).T).astype(np.float32)
    bp_eff = (b_proj + b_attn[2 * C:] @ w_proj).reshape(1, C).astype(np.float16)

    if "nc" not in _CACHE:
        _CACHE["nc"] = build_bass()
    nc = _CACHE["nc"]

    shared = {"waq": waq16, "wav": wav16, "wp": wp16, "bq": bq, "bp": bp_eff}
    in_maps = [dict(shared, xT=xT[b]) for b in range(B)]
    if _trace:
        _install_ntff_hook()
    res = run_bass_kernel_spmd(
        nc, in_maps, list(range(B)), trace=_trace, tmpdir=_tmpdir
    )
    out = np.stack([res.results[b]["y"].astype(np.float32) for b in range(B)],
                   axis=0)
    if _trace:
        return out, res
    return out


# revision 22
# speedup vs baseline: 1.0559x; 1.0559x over previous
"""Trainium2 Bass kernel for CausalSelfAttention (B=8, N=1024, C=768, H=12).

Sharding: data-parallel over batch - one batch element per NeuronCore,
weights replicated, no collectives.

v2 design (vs the 223us baseline):
  - x is transposed and fp8-pair-packed on the HOST (free), killing all
    on-chip PE transposes and their DVE copybacks.
  - fp8 budget: only es (exp output) and the packed v tiles are fp8 -
    each fp8 tensor feeding a zero-mean contraction costs ~1.2% final
    rel error (no averaging benefit), so the stack must stay under the
    2e-2 gate: es+v fp8 = ~1.7%, adding qk fp8 would tip it to ~2.2%.
    q/k, v-compute, and projection matmuls stay fp16.
  - score matmuls (K=64) run two heads concurrently in the PE array via
    tile_position row packing (rows 0-63 / 64-127).
  - exp runs on ACT straight to fp8 es tiles laid out as DoubleRow kt
    pairs; PV runs fp8 DoubleRow over 256-token contractions with the
    rowsum ones-column folded into the packed v tiles.
  - bias algebra: k-bias cancels in softmax (terms depend only on q);
    v-bias is folded into an effective b_proj on the host; only the
    q-bias survives on-chip (6 DVE tensor-scalar adds).
  - normalization: reciprocal of the PV rowsum row, DRAM-bounce
    partition-broadcast, one fp16 multiply per chunk, trailing the
    pipeline by a head pair.
  - output projection split: contraction chunks 0-4 run inside the
    attention phase once heads 0-9 are normalized; only the last chunk
    (heads 10/11) plus a DVE add remains as the serial tail.
"""

import sys
import types

import numpy as np
import ml_dtypes

import concourse.bass as bass
import concourse.tile as tile
from concourse import bacc
from concourse import mybir

F32 = mybir.dt.float32
F16 = mybir.dt.float16
F8 = mybir.dt.float8e4
AF = mybir.ActivationFunctionType
DR = mybir.MatmulPerfMode.DoubleRow

B, N, C, H, D = 8, 1024, 768, 12, 64
CK = C // 128        # 6 fp16 contraction chunks
JK = C // 256        # 3 DoubleRow contraction chunks
NT = N // 128        # 8 token tiles
QC = N // 512        # 2 moving chunks of 512 tokens
NP = NT // 2         # 4 kt pair tiles for DoubleRow PV
VW = 80              # padded per-head width in packed v tiles (stride %16)
SCALE = 1.0 / np.sqrt(D)
USE_DR_PV = True


def _install_ntff_hook():
    """Register the axon NTFF profiling hook if the image's antenv lacks it."""
    try:
        from antenv.axon_hooks import get_axon_ntff_profile_hook  # noqa: F401
        return
    except ImportError:
        pass
    try:
        import antenv
        mod = types.ModuleType("antenv.axon_hooks")
        _h = [None]
        mod.set_axon_ntff_profile_hook = lambda h: _h.__setitem__(0, h)
        mod.get_axon_ntff_profile_hook = lambda: _h[0]
        antenv.axon_hooks = mod
        sys.modules["antenv.axon_hooks"] = mod
        if "/root/.axon_site" not in sys.path:
            sys.path.insert(0, "/root/.axon_site")
        from trn_agent_boot.trn_boot import _ntff_profile_via_ctypes
        hook = _ntff_profile_via_ctypes("/opt/axon/libaxon_pjrt.so")
        if hook is not None:
            mod.set_axon_ntff_profile_hook(hook)
    except Exception:
        pass


def build_bass(debug=False):
    nc = bacc.Bacc("TRN2", target_bir_lowering=False, debug=False)
    xT = nc.dram_tensor("xT", [CK, 128, N], F16, kind="ExternalInput").ap()
    waq = nc.dram_tensor("waq", [CK, 128, 2 * C], F16, kind="ExternalInput").ap()
    wav = nc.dram_tensor("wav", [CK, 128, C], F16, kind="ExternalInput").ap()
    wp = nc.dram_tensor("wp", [CK, 128, C], F16, kind="ExternalInput").ap()
    bq = nc.dram_tensor("bq", [128, CK], F32, kind="ExternalInput").ap()
    bp = nc.dram_tensor("bp", [1, C], F16, kind="ExternalInput").ap()
    y = nc.dram_tensor("y", [N, C], F16, kind="ExternalOutput").ap()
    dbg = None
    if debug:
        dbg = {
            "dbg_qT": nc.dram_tensor("dbg_qT", [CK, 128, N], F16, kind="ExternalOutput").ap(),
            "dbg_kT": nc.dram_tensor("dbg_kT", [CK, 128, N], F16, kind="ExternalOutput").ap(),
            "dbg_ao": nc.dram_tensor("dbg_ao", [CK, 128, N], F16, kind="ExternalOutput").ap(),
            "dbg_vp": nc.dram_tensor("dbg_vp", [NP, 128, 2, H, D + 1], F8, kind="ExternalOutput").ap(),
            "dbg_esa": nc.dram_tensor("dbg_esa", [128, NT, N], F8, kind="ExternalOutput").ap(),
            "dbg_esb": nc.dram_tensor("dbg_esb", [128, NT, N], F8, kind="ExternalOutput").ap(),
            "dbg_pvc": nc.dram_tensor("dbg_pvc", [QC, D + 1, 512], F32, kind="ExternalOutput").ap(),
            "dbg_rsi": nc.dram_tensor("dbg_rsi", [QC, 1, 512], F32, kind="ExternalOutput").ap(),
            "dbg_rbc": nc.dram_tensor("dbg_rbc", [QC, D, 512], F32, kind="ExternalOutput").ap(),
        }

    with tile.TileContext(nc) as tc:
        build_body(nc, tc, xT, waq, wav, wp, bq, bp, y, dbg)
    nc.compile()
    return nc


def build_body(nc, tc, xT_d, waq_d, wav_d, wp_d, bq_d, bp_d, y_d, dbg=None):
    from contextlib import ExitStack

    ctx = ExitStack()
    with ctx:
        singles = ctx.enter_context(tc.tile_pool(name="singles", bufs=1))
        persist = ctx.enter_context(tc.tile_pool(name="persist", bufs=1))
        p_e = ctx.enter_context(tc.tile_pool(name="exps", bufs=3))
        p_n = ctx.enter_context(tc.tile_pool(name="norm", bufs=4))
        p_ys = ctx.enter_context(tc.tile_pool(name="ysb", bufs=2))
        # PSUM budget (8 banks): scores 2 tags x 1 buf x 2 banks = 4
        # (A/B tag alternation keeps ACT saturated without double-buffering),
        # pv 2 bufs x 1 bank = 2, filler "mm" [128,512] 2 bufs x 1 bank = 2
        # (double-buffered so PE filler is not serialized on DVE drains).
        p_sc = ctx.enter_context(tc.tile_pool(name="scpsum", bufs=1, space="PSUM"))
        p_pv = ctx.enter_context(tc.tile_pool(name="pvpsum", bufs=2, space="PSUM"))
        p_mm = ctx.enter_context(tc.tile_pool(name="mmpsum", bufs=2, space="PSUM"))
        p_dr = ctx.enter_context(tc.tile_pool(name="drscratch", bufs=4, space="DRAM"))

        ones = singles.tile([1, 128], F16, tag="ones")
        nc.gpsimd.memset(ones[:], 1.0)
        b_q = singles.tile([128, CK], F32, tag="b_q")
        nc.sync.dma_start(out=b_q[:], in_=bq_d[:, :])
        bp_row = singles.tile([1, C], F16, tag="bp_row")
        nc.gpsimd.dma_start(out=bp_row[:], in_=bp_d[:, :])

        # persistent SBUF tensors
        xT = [persist.tile([128, N], F16, name=f"xT{i}", tag=f"xT{i}") for i in range(CK)]
        waq = [persist.tile([128, 2 * C], F16, name=f"waq{i}", tag=f"waq{i}") for i in range(CK)]
        wav = [persist.tile([128, C], F16, name=f"wav{i}", tag=f"wav{i}") for i in range(CK)]
        wps = [persist.tile([128, C], F16, name=f"wp{i}", tag=f"wp{i}") for i in range(CK)]
        qT = [persist.tile([128, N], F16, name=f"qT{i}", tag=f"qT{i}") for i in range(CK)]
        kT = [persist.tile([128, N], F16, name=f"kT{i}", tag=f"kT{i}") for i in range(CK)]
        vpk = [persist.tile([128, 2, H, VW], F8, name=f"vp{j}", tag=f"vp{j}") for j in range(NP)]
        aout = [persist.tile([128, N], F16, name=f"ao{i}", tag=f"ao{i}") for i in range(CK)]
        ya = [persist.tile([128, C], F16, name=f"ya{t}", tag=f"ya{t}") for t in range(NT)]

        # input DMAs (front-load the ones the ramp needs)
        for i in range(CK):
            nc.sync.dma_start(out=xT[i][:], in_=xT_d[i])
            nc.sync.dma_start(out=waq[i][:], in_=waq_d[i])
        for i in range(CK):
            nc.scalar.dma_start(out=wav[i][:], in_=wav_d[i])
        for i in range(CK):
            nc.gpsimd.dma_start(out=wps[i][:], in_=wp_d[i])
        for j in range(NP):
            nc.gpsimd.memset(vpk[j][:, :, :, D:D + 1], 1.0)

        # ~2.5us of tiny matmuls to warm the PE HAM while input DMAs land
        ident = singles.tile([128, 128], F16, tag="ident")
        nc.gpsimd.memset(ident[:], 0.0)
        for _ in range(30):
            wp_ps = p_mm.tile([128, 512], F32, name="warm", tag="mm")
            nc.tensor.matmul(wp_ps[:, 0:128], ident[:], ident[:], start=True, stop=True)

        def emit_qk(m):
            """q^T (m<6) or k^T (m>=6) chunk m: [128, 1024] fp16."""
            for qc in range(QC):
                p = p_mm.tile([128, 512], F32, name="qkp", tag="mm")
                for ci in range(CK):
                    nc.tensor.matmul(
                        p[:],
                        waq[ci][:, m * 128:(m + 1) * 128],
                        xT[ci][:, qc * 512:(qc + 1) * 512],
                        start=(ci == 0),
                        stop=(ci == CK - 1),
                    )
                sl = slice(qc * 512, (qc + 1) * 512)
                if m < CK:
                    nc.vector.tensor_scalar_add(qT[m][:, sl], p[:], b_q[:, m:m + 1])
                else:
                    nc.vector.tensor_copy(out=kT[m - CK][:, sl], in_=p[:])

        def emit_v_tile(t):
            """v rows for token tile t -> packed fp8 pair tile (fp16 matmul)."""
            for off, w in ((0, 512), (512, 256)):
                p = p_mm.tile([128, 512], F32, name="vp", tag="mm")
                for ci in range(CK):
                    nc.tensor.matmul(
                        p[:, 0:w],
                        xT[ci][:, t * 128:(t + 1) * 128],
                        wav[ci][:, off:off + w],
                        start=(ci == 0),
                        stop=(ci == CK - 1),
                    )
                nc.vector.tensor_copy(
                    out=vpk[t // 2][:, t % 2, off // D:(off + w) // D, 0:D],
                    in_=p[:, 0:w].rearrange("p (h d) -> p h d", d=D),
                )

        def emit_scores_kt(pair, kt, es_a, es_b):
            """Row-packed scores for heads (2*pair, 2*pair+1) at key tile kt."""
            hq, hk = qT[pair], kT[pair]
            for half, es in ((0, es_a), (1, es_b)):
                ps = p_sc.tile([128, 1024], F32, name=f"sc{half}", tag=f"sc{half}")
                lo = half * D
                for qc in range(QC):
                    nc.tensor.matmul(
                        ps[:, qc * 512:(qc + 1) * 512],
                        hk[lo:lo + D, kt * 128:(kt + 1) * 128],
                        hq[lo:lo + D, qc * 512:(qc + 1) * 512],
                        start=True,
                        stop=True,
                        tile_position=(lo, 0),
                    )
                nc.scalar.activation(
                    out=es[:, kt, :], in_=ps[:], func=AF.Exp, scale=float(SCALE)
                )

        def emit_pv_j(h, es, pvs, j):
            """Accumulate PV pair j for head h into pvs[qc]."""
            for qc in range(QC):
                if USE_DR_PV:
                    nc.tensor.matmul(
                        pvs[qc][:],
                        vpk[j][:, :, h, 0:D + 1],
                        es[:, 2 * j:2 * j + 2, qc * 512:(qc + 1) * 512],
                        start=(j == 0),
                        stop=(j == NP - 1),
                        perf_mode=DR,
                    )
                else:
                    for i in range(2):
                        nc.tensor.matmul(
                            pvs[qc][:],
                            vpk[j][:, i, h, 0:D + 1],
                            es[:, 2 * j + i, qc * 512:(qc + 1) * 512],
                            start=(j == 0 and i == 0),
                            stop=(j == NP - 1 and i == 1),
                        )

        def emit_pv_drain(h, pvs):
            """Copy PV out of PSUM, start the rowsum-reciprocal DRAM bounce."""
            out = []
            for qc in range(QC):
                pv = pvs[qc]
                # reciprocal_approx_fast is a bitwise custom DVE op: on
                # silicon it can neither read PSUM nor shift base partitions,
                # so the raw rowsum row bounces through DRAM (partition 64 ->
                # broadcast on 0..63) and the reciprocal runs aligned later.
                pvc = p_n.tile([D + 1, 512], F32, name="pvc", tag="pvc", bufs=6)
                nc.vector.tensor_copy(out=pvc[:], in_=pv[0:D + 1, :])
                rs_d = p_dr.tile([1, 512], F32, name="rs_d", tag="rs_d")
                nc.sync.dma_start(out=rs_d[:], in_=pvc[D:D + 1, :])
                rbc_raw = p_n.tile([D, 512], F32, name="rbcr", tag="rbcr", bufs=4)
                nc.sync.dma_start(out=rbc_raw[:], in_=rs_d[0, :].partition_broadcast(D))
                out.append((pvc, rbc_raw))
            return out

        def emit_norm_mul(h, drained):
            for qc in range(QC):
                pvc, rbc_raw = drained[qc]
                rbc = p_n.tile([D, 512], F32, name="rbc", tag="rbc", bufs=4)
                nc.vector.reciprocal_approx_fast(out=rbc[:], in_=rbc_raw[:])
                if dbg is not None and h == H - 1:
                    nc.sync.dma_start(out=dbg["dbg_pvc"][qc], in_=pvc[0:D + 1, :])
                    nc.sync.dma_start(out=dbg["dbg_rbc"][qc], in_=rbc[:])
                nc.gpsimd.tensor_mul(
                    aout[h // 2][(h % 2) * D:(h % 2) * D + D,
                                 qc * 512:(qc + 1) * 512],
                    pvc[0:D, :],
                    rbc[:],
                )

        def emit_proj_a(t):
            """Output projection over contraction chunks 0-4 (+bias) for tile t."""
            for off, w in ((0, 512), (512, 256)):
                p = p_mm.tile([128, 512], F32, name="ypa", tag="mm")
                for ci in range(CK - 1):
                    nc.tensor.matmul(
                        p[:, 0:w],
                        aout[ci][:, t * 128:(t + 1) * 128],
                        wps[ci][:, off:off + w],
                        start=(ci == 0),
                        stop=False,
                    )
                nc.tensor.matmul(
                    p[:, 0:w],
                    ones[0:1, 0:128],
                    bp_row[0:1, off:off + w],
                    start=False,
                    stop=True,
                )
                nc.vector.tensor_copy(out=ya[t][:, off:off + w], in_=p[:, 0:w])

        def emit_proj_b(t):
            """Last contraction chunk (heads 10/11), add to ya, store y."""
            ysb = p_ys.tile([128, C], F16, tag="ysb")
            for off, w in ((0, 512), (512, 256)):
                p = p_mm.tile([128, 512], F32, name="ypb", tag="mm")
                nc.tensor.matmul(
                    p[:, 0:w],
                    aout[CK - 1][:, t * 128:(t + 1) * 128],
                    wps[CK - 1][:, off:off + w],
                    start=True,
                    stop=True,
                )
                nc.vector.tensor_add(ysb[:, off:off + w], ya[t][:, off:off + w],
                                     p[:, 0:w])
            nc.sync.dma_start(out=y_d[t * 128:(t + 1) * 128, :], in_=ysb[:])

        # ---------------- pipeline ----------------
        # filler: closures the PE chews on while ACT streams exps
        filler = [(lambda m=m: emit_qk(m)) for m in (0, CK)]
        filler += [(lambda t=t: emit_v_tile(t)) for t in (0, 1)]
        for q in filler:
            q()

        filler = []
        filler += [(lambda t=t: emit_v_tile(t)) for t in range(2, NT)]
        for i in range(1, H // 2):
            filler.append(lambda m=i: emit_qk(m))
            filler.append(lambda m=CK + i: emit_qk(m))

        pending_mul = []   # (head, drained) waiting a safe distance
        pending_pv_b = []  # head B pv closures, run early next iter

        def filler_step(k=1):
            for _ in range(k):
                if pending_mul:
                    h, dr = pending_mul.pop(0)
                    emit_norm_mul(h, dr)
                    continue
                if filler:
                    filler.pop(0)()

        for pair in range(H // 2):
            hA, hB = 2 * pair, 2 * pair + 1
            es_a = p_e.tile([128, NT, N], F8, name="esa", tag="es")
            es_b = p_e.tile([128, NT, N], F8, name="esb", tag="es")
            pvsA = None
            for kt in range(NT):
                emit_scores_kt(pair, kt, es_a, es_b)
                if kt == 0 and pending_pv_b:
                    # previous pair's head B: es complete, banks now free
                    pending_pv_b.pop(0)()
                if kt % 2 == 1:
                    j = kt // 2
                    if j == 0:
                        pvsA = [p_pv.tile([D + 1, 512], F32, name=f"pva{qc}", tag="pv")
                                for qc in range(QC)]
                    emit_pv_j(hA, es_a, pvsA, j)
                    filler_step(1)
                else:
                    filler_step(1)
            pending_mul.append((hA, emit_pv_drain(hA, pvsA)))

            def run_pv_b(h=hB, es=es_b):
                pvs = [p_pv.tile([D + 1, 512], F32, name=f"pvb{qc}", tag="pv")
                       for qc in range(QC)]
                for j in range(NP):
                    emit_pv_j(h, es, pvs, j)
                pending_mul.append((h, emit_pv_drain(h, pvs)))
            pending_pv_b.append(run_pv_b)

            if pair == H // 2 - 1:
                pending_pv_b.pop(0)()
            if pair == H // 2 - 2:
                # after this pair's muls run, heads 0-9 will be complete;
                # queue proj chunk A behind them
                filler += [(lambda t=t: emit_proj_a(t)) for t in range(NT)]

        # drain the tail: remaining muls (heads 10, 11), proj A remainder, proj B
        while pending_mul or filler:
            filler_step(1)
        for t in range(NT):
            emit_proj_b(t)

        if dbg is not None:
            for i in range(CK):
                nc.sync.dma_start(out=dbg["dbg_qT"][i], in_=qT[i][:])
                nc.sync.dma_start(out=dbg["dbg_kT"][i], in_=kT[i][:])
                nc.sync.dma_start(out=dbg["dbg_ao"][i], in_=aout[i][:])
            for j in range(NP):
                nc.sync.dma_start(out=dbg["dbg_vp"][j], in_=vpk[j][:, :, :, 0:D + 1])
            nc.sync.dma_start(out=dbg["dbg_esa"], in_=es_a[:])
            nc.sync.dma_start(out=dbg["dbg_esb"], in_=es_b[:])


_CACHE = {}


def kernel(x, pad_mask=None, w_attn=None, b_attn=None, w_proj=None, b_proj=None,
           _trace=False, _tmpdir=None):
    from concourse.bass_utils import run_bass_kernel_spmd

    FP8 = ml_dtypes.float8_e4m3

    x = np.asarray(x, dtype=np.float32)
    w_attn = np.asarray(w_attn, dtype=np.float32)
    b_attn = np.asarray(b_attn, dtype=np.float32)
    w_proj = np.asarray(w_proj, dtype=np.float32)
    b_proj = np.asarray(b_proj, dtype=np.float32)

    # host-side packing (free): transpose x
    xT = np.ascontiguousarray(
        x.transpose(0, 2, 1).reshape(B, CK, 128, N).astype(np.float16))
    waq16 = np.ascontiguousarray(
        w_attn[:, :2 * C].reshape(CK, 128, 2 * C)).astype(np.float16)
    wav16 = np.ascontiguousarray(
        w_attn[:, 2 * C:].reshape(CK, 128, C)).astype(np.float16)
    wp16 = np.ascontiguousarray(w_proj.reshape(CK, 128, C)).astype(np.float16)
    bq = np.ascontiguousarray(b_attn[:C].reshape(CK, 128).T).astype(np.float32)
    bp_eff = (b_proj + b_attn[2 * C:] @ w_proj).reshape(1, C).astype(np.float16)

    if "nc" not in _CACHE:
        _CACHE["nc"] = build_bass()
    nc = _CACHE["nc"]

    shared = {"waq": waq16, "wav": wav16, "wp": wp16, "bq": bq, "bp": bp_eff}
    in_maps = [dict(shared, xT=xT[b]) for b in range(B)]
    if _trace:
        _install_ntff_hook()
    res = run_bass_kernel_spmd(
        nc, in_maps, list(range(B)), trace=_trace, tmpdir=_tmpdir
    )
    out = np.stack([res.results[b]["y"].astype(np.float32) for b in range(B)],
                   axis=0)
    if _trace:
        return out, res
    return out


# revision 24
# speedup vs baseline: 1.1190x; 1.0598x over previous
"""Trainium2 Bass kernel for CausalSelfAttention (B=8, N=1024, C=768, H=12).

Sharding: data-parallel over batch - one batch element per NeuronCore,
weights replicated, no collectives.

v2 design (vs the 223us baseline):
  - x is transposed and fp8-pair-packed on the HOST (free), killing all
    on-chip PE transposes and their DVE copybacks.
  - fp8 budget: only es (exp output) and the packed v tiles are fp8 -
    each fp8 tensor feeding a zero-mean contraction costs ~1.2% final
    rel error (no averaging benefit), so the stack must stay under the
    2e-2 gate: es+v fp8 = ~1.7%, adding qk fp8 would tip it to ~2.2%.
    q/k, v-compute, and projection matmuls stay fp16.
  - score matmuls (K=64) run two heads concurrently in the PE array via
    tile_position row packing (rows 0-63 / 64-127).
  - exp runs on ACT straight to fp8 es tiles laid out as DoubleRow kt
    pairs; PV runs fp8 DoubleRow over 256-token contractions with the
    rowsum ones-column folded into the packed v tiles.
  - bias algebra: k-bias cancels in softmax (terms depend only on q);
    v-bias is folded into an effective b_proj on the host; only the
    q-bias survives on-chip (6 DVE tensor-scalar adds).
  - normalization: reciprocal of the PV rowsum row, DRAM-bounce
    partition-broadcast, one fp16 multiply per chunk, trailing the
    pipeline by a head pair.
  - output projection split: contraction chunks 0-4 run inside the
    attention phase once heads 0-9 are normalized; only the last chunk
    (heads 10/11) plus a DVE add remains as the serial tail.
"""

import sys
import types

import numpy as np
import ml_dtypes

import concourse.bass as bass
import concourse.tile as tile
from concourse import bacc
from concourse import mybir

F32 = mybir.dt.float32
F16 = mybir.dt.float16
F8 = mybir.dt.float8e4
AF = mybir.ActivationFunctionType
DR = mybir.MatmulPerfMode.DoubleRow

B, N, C, H, D = 8, 1024, 768, 12, 64
CK = C // 128        # 6 fp16 contraction chunks
JK = C // 256        # 3 DoubleRow contraction chunks
NT = N // 128        # 8 token tiles
QC = N // 512        # 2 moving chunks of 512 tokens
NP = NT // 2         # 4 kt pair tiles for DoubleRow PV
VW = 80              # padded per-head width in packed v tiles (stride %16)
SCALE = 1.0 / np.sqrt(D)
USE_DR_PV = True


def _install_ntff_hook():
    """Register the axon NTFF profiling hook if the image's antenv lacks it."""
    try:
        from antenv.axon_hooks import get_axon_ntff_profile_hook  # noqa: F401
        return
    except ImportError:
        pass
    try:
        import antenv
        mod = types.ModuleType("antenv.axon_hooks")
        _h = [None]
        mod.set_axon_ntff_profile_hook = lambda h: _h.__setitem__(0, h)
        mod.get_axon_ntff_profile_hook = lambda: _h[0]
        antenv.axon_hooks = mod
        sys.modules["antenv.axon_hooks"] = mod
        if "/root/.axon_site" not in sys.path:
            sys.path.insert(0, "/root/.axon_site")
        from trn_agent_boot.trn_boot import _ntff_profile_via_ctypes
        hook = _ntff_profile_via_ctypes("/opt/axon/libaxon_pjrt.so")
        if hook is not None:
            mod.set_axon_ntff_profile_hook(hook)
    except Exception:
        pass


def build_bass(debug=False):
    nc = bacc.Bacc("TRN2", target_bir_lowering=False, debug=False)
    xT = nc.dram_tensor("xT", [CK, 128, N], F16, kind="ExternalInput").ap()
    waq = nc.dram_tensor("waq", [CK, 128, 2 * C], F16, kind="ExternalInput").ap()
    wav = nc.dram_tensor("wav", [CK, 128, C], F16, kind="ExternalInput").ap()
    wp = nc.dram_tensor("wp", [CK, 128, C], F16, kind="ExternalInput").ap()
    bq = nc.dram_tensor("bq", [128, CK], F32, kind="ExternalInput").ap()
    bp = nc.dram_tensor("bp", [1, C], F16, kind="ExternalInput").ap()
    y = nc.dram_tensor("y", [N, C], F16, kind="ExternalOutput").ap()
    dbg = None
    if debug:
        dbg = {
            "dbg_qT": nc.dram_tensor("dbg_qT", [CK, 128, N], F16, kind="ExternalOutput").ap(),
            "dbg_kT": nc.dram_tensor("dbg_kT", [CK, 128, N], F16, kind="ExternalOutput").ap(),
            "dbg_ao": nc.dram_tensor("dbg_ao", [CK, 128, N], F16, kind="ExternalOutput").ap(),
            "dbg_vp": nc.dram_tensor("dbg_vp", [NP, 128, 2, H, D + 1], F8, kind="ExternalOutput").ap(),
            "dbg_esa": nc.dram_tensor("dbg_esa", [128, NT, N], F8, kind="ExternalOutput").ap(),
            "dbg_esb": nc.dram_tensor("dbg_esb", [128, NT, N], F8, kind="ExternalOutput").ap(),
            "dbg_pvc": nc.dram_tensor("dbg_pvc", [QC, D + 1, 512], F32, kind="ExternalOutput").ap(),
            "dbg_rsi": nc.dram_tensor("dbg_rsi", [QC, 1, 512], F32, kind="ExternalOutput").ap(),
            "dbg_rbc": nc.dram_tensor("dbg_rbc", [QC, D, 512], F32, kind="ExternalOutput").ap(),
        }

    with tile.TileContext(nc) as tc:
        build_body(nc, tc, xT, waq, wav, wp, bq, bp, y, dbg)
    nc.compile()
    return nc


def build_body(nc, tc, xT_d, waq_d, wav_d, wp_d, bq_d, bp_d, y_d, dbg=None):
    from contextlib import ExitStack

    ctx = ExitStack()
    with ctx:
        singles = ctx.enter_context(tc.tile_pool(name="singles", bufs=1))
        persist = ctx.enter_context(tc.tile_pool(name="persist", bufs=1))
        p_e = ctx.enter_context(tc.tile_pool(name="exps", bufs=3))
        p_n = ctx.enter_context(tc.tile_pool(name="norm", bufs=4))
        p_ys = ctx.enter_context(tc.tile_pool(name="ysb", bufs=2))
        # PSUM budget (8 banks): scores 2 tags x 1 buf x 2 banks = 4
        # (A/B tag alternation keeps ACT saturated without double-buffering),
        # pv 2 bufs x 1 bank = 2, filler "mm" [128,512] 2 bufs x 1 bank = 2
        # (double-buffered so PE filler is not serialized on DVE drains).
        p_sc = ctx.enter_context(tc.tile_pool(name="scpsum", bufs=1, space="PSUM"))
        p_pv = ctx.enter_context(tc.tile_pool(name="pvpsum", bufs=2, space="PSUM"))
        p_mm = ctx.enter_context(tc.tile_pool(name="mmpsum", bufs=2, space="PSUM"))
        p_dr = ctx.enter_context(tc.tile_pool(name="drscratch", bufs=4, space="DRAM"))

        ones = singles.tile([1, 128], F16, tag="ones")
        nc.gpsimd.memset(ones[:], 1.0)
        # warm the PE HAM immediately and preload the exp table set while
        # input DMAs are still in flight
        ident = singles.tile([128, 128], F16, tag="ident")
        nc.vector.memset(ident[:], 0.0)
        for _ in range(40):
            wp_ps = p_mm.tile([128, 512], F32, name="warm", tag="mm")
            nc.tensor.matmul(wp_ps[:, 0:128], ident[:], ident[:], start=True, stop=True)
        tdummy = singles.tile([1, 16], F16, tag="tdummy")
        nc.scalar.activation(out=tdummy[:], in_=ident[0:1, 0:16], func=AF.Exp,
                             scale=1.0)
        b_q = singles.tile([128, CK], F32, tag="b_q")
        nc.sync.dma_start(out=b_q[:], in_=bq_d[:, :])
        bp_row = singles.tile([1, C], F16, tag="bp_row")
        nc.gpsimd.dma_start(out=bp_row[:], in_=bp_d[:, :])

        # persistent SBUF tensors
        xT = [persist.tile([128, N], F16, name=f"xT{i}", tag=f"xT{i}") for i in range(CK)]
        waq = [persist.tile([128, 2 * C], F16, name=f"waq{i}", tag=f"waq{i}") for i in range(CK)]
        wav = [persist.tile([128, C], F16, name=f"wav{i}", tag=f"wav{i}") for i in range(CK)]
        wps = [persist.tile([128, C], F16, name=f"wp{i}", tag=f"wp{i}") for i in range(CK)]
        qT = [persist.tile([128, N], F16, name=f"qT{i}", tag=f"qT{i}") for i in range(CK)]
        kT = [persist.tile([128, N], F16, name=f"kT{i}", tag=f"kT{i}") for i in range(CK)]
        vpk = [persist.tile([128, 2, H, VW], F8, name=f"vp{j}", tag=f"vp{j}") for j in range(NP)]
        aout = [persist.tile([128, N], F16, name=f"ao{i}", tag=f"ao{i}") for i in range(CK)]
        ya = [persist.tile([128, C], F16, name=f"ya{t}", tag=f"ya{t}") for t in range(NT)]

        # input DMAs (front-load the ones the ramp needs)
        for i in range(CK):
            nc.sync.dma_start(out=xT[i][:], in_=xT_d[i])
            nc.sync.dma_start(out=waq[i][:], in_=waq_d[i])
        for i in range(CK):
            nc.scalar.dma_start(out=wav[i][:], in_=wav_d[i])
        for j in range(NP):
            nc.gpsimd.memset(vpk[j][:, :, :, D:D + 1], 1.0)
        for i in range(CK):
            nc.gpsimd.dma_start(out=wps[i][:], in_=wp_d[i])

        def emit_qk(m):
            """q^T (m<6) or k^T (m>=6) chunk m: [128, 1024] fp16."""
            for qc in range(QC):
                p = p_mm.tile([128, 512], F32, name="qkp", tag="mm")
                for ci in range(CK):
                    nc.tensor.matmul(
                        p[:],
                        waq[ci][:, m * 128:(m + 1) * 128],
                        xT[ci][:, qc * 512:(qc + 1) * 512],
                        start=(ci == 0),
                        stop=(ci == CK - 1),
                    )
                sl = slice(qc * 512, (qc + 1) * 512)
                if m < CK:
                    nc.vector.tensor_scalar_add(qT[m][:, sl], p[:], b_q[:, m:m + 1])
                else:
                    nc.vector.tensor_copy(out=kT[m - CK][:, sl], in_=p[:])

        def emit_v_tile(t):
            """v rows for token tile t -> packed fp8 pair tile (fp16 matmul)."""
            for off, w in ((0, 512), (512, 256)):
                p = p_mm.tile([128, 512], F32, name="vp", tag="mm")
                for ci in range(CK):
                    nc.tensor.matmul(
                        p[:, 0:w],
                        xT[ci][:, t * 128:(t + 1) * 128],
                        wav[ci][:, off:off + w],
                        start=(ci == 0),
                        stop=(ci == CK - 1),
                    )
                nc.vector.tensor_copy(
                    out=vpk[t // 2][:, t % 2, off // D:(off + w) // D, 0:D],
                    in_=p[:, 0:w].rearrange("p (h d) -> p h d", d=D),
                )

        def emit_scores_kt(pair, kt, es_a, es_b):
            """Row-packed scores for heads (2*pair, 2*pair+1) at key tile kt."""
            hq, hk = qT[pair], kT[pair]
            for half, es in ((0, es_a), (1, es_b)):
                ps = p_sc.tile([128, 1024], F32, name=f"sc{half}", tag=f"sc{half}")
                lo = half * D
                for qc in range(QC):
                    nc.tensor.matmul(
                        ps[:, qc * 512:(qc + 1) * 512],
                        hk[lo:lo + D, kt * 128:(kt + 1) * 128],
                        hq[lo:lo + D, qc * 512:(qc + 1) * 512],
                        start=True,
                        stop=True,
                        tile_position=(lo, 0),
                    )
                nc.scalar.activation(
                    out=es[:, kt, :], in_=ps[:], func=AF.Exp, scale=float(SCALE)
                )

        def emit_pv_j(h, es, pvs, j):
            """Accumulate PV pair j for head h into pvs[qc]."""
            for qc in range(QC):
                if USE_DR_PV:
                    nc.tensor.matmul(
                        pvs[qc][:],
                        vpk[j][:, :, h, 0:D + 1],
                        es[:, 2 * j:2 * j + 2, qc * 512:(qc + 1) * 512],
                        start=(j == 0),
                        stop=(j == NP - 1),
                        perf_mode=DR,
                    )
                else:
                    for i in range(2):
                        nc.tensor.matmul(
                            pvs[qc][:],
                            vpk[j][:, i, h, 0:D + 1],
                            es[:, 2 * j + i, qc * 512:(qc + 1) * 512],
                            start=(j == 0 and i == 0),
                            stop=(j == NP - 1 and i == 1),
                        )

        def emit_pv_drain(h, pvs):
            """Copy PV out of PSUM, start the rowsum-reciprocal DRAM bounce."""
            out = []
            for qc in range(QC):
                pv = pvs[qc]
                # reciprocal_approx_fast is a bitwise custom DVE op: on
                # silicon it can neither read PSUM nor shift base partitions,
                # so the raw rowsum row bounces through DRAM (partition 64 ->
                # broadcast on 0..63) and the reciprocal runs aligned later.
                pvc = p_n.tile([D + 1, 512], F32, name="pvc", tag="pvc", bufs=6)
                nc.vector.tensor_copy(out=pvc[:], in_=pv[0:D + 1, :])
                rs_d = p_dr.tile([1, 512], F32, name="rs_d", tag="rs_d")
                nc.sync.dma_start(out=rs_d[:], in_=pvc[D:D + 1, :])
                rbc_raw = p_n.tile([D, 512], F32, name="rbcr", tag="rbcr", bufs=4)
                nc.sync.dma_start(out=rbc_raw[:], in_=rs_d[0, :].partition_broadcast(D))
                out.append((pvc, rbc_raw))
            return out

        def emit_norm_mul(h, drained):
            for qc in range(QC):
                pvc, rbc_raw = drained[qc]
                rbc = p_n.tile([D, 512], F32, name="rbc", tag="rbc", bufs=4)
                nc.vector.reciprocal_approx_fast(out=rbc[:], in_=rbc_raw[:])
                if dbg is not None and h == H - 1:
                    nc.sync.dma_start(out=dbg["dbg_pvc"][qc], in_=pvc[0:D + 1, :])
                    nc.sync.dma_start(out=dbg["dbg_rbc"][qc], in_=rbc[:])
                nc.gpsimd.tensor_mul(
                    aout[h // 2][(h % 2) * D:(h % 2) * D + D,
                                 qc * 512:(qc + 1) * 512],
                    pvc[0:D, :],
                    rbc[:],
                )

        def emit_proj_a(t):
            """Output projection over contraction chunks 0-3 (+bias) for tile t."""
            for off, w in ((0, 512), (512, 256)):
                p = p_mm.tile([128, 512], F32, name="ypa", tag="mm")
                for ci in range(CK - 2):
                    nc.tensor.matmul(
                        p[:, 0:w],
                        aout[ci][:, t * 128:(t + 1) * 128],
                        wps[ci][:, off:off + w],
                        start=(ci == 0),
                        stop=False,
                    )
                nc.tensor.matmul(
                    p[:, 0:w],
                    ones[0:1, 0:128],
                    bp_row[0:1, off:off + w],
                    start=False,
                    stop=True,
                )
                nc.vector.tensor_copy(out=ya[t][:, off:off + w], in_=p[:, 0:w])

        def emit_proj_b(t):
            """Last two contraction chunks (heads 8-11), add to ya, store y."""
            ysb = p_ys.tile([128, C], F16, tag="ysb")
            for off, w in ((0, 512), (512, 256)):
                p = p_mm.tile([128, 512], F32, name="ypb", tag="mm")
                for ci in (CK - 2, CK - 1):
                    nc.tensor.matmul(
                        p[:, 0:w],
                        aout[ci][:, t * 128:(t + 1) * 128],
                        wps[ci][:, off:off + w],
                        start=(ci == CK - 2),
                        stop=(ci == CK - 1),
                    )
                nc.vector.tensor_add(ysb[:, off:off + w], ya[t][:, off:off + w],
                                     p[:, 0:w])
            nc.sync.dma_start(out=y_d[t * 128:(t + 1) * 128, :], in_=ysb[:])

        # ---------------- pipeline ----------------
        # The scores->exp chain paces the kernel (ACT-bound); everything else
        # is spread across the 8 kt "slots" of each head pair so the PE never
        # idles long enough for the HAM to re-throttle its clock.
        emit_qk(0)
        emit_qk(CK)

        filler = [(lambda t=t: emit_v_tile(t)) for t in range(NT)]
        for i in range(1, H // 2):
            filler.append(lambda m=i: emit_qk(m))
            filler.append(lambda m=CK + i: emit_qk(m))

        urgent = []        # pv_b j-chunks of the previous pair, one per slot
        pending_mul = []   # (head, drained) trailing the DRAM bounce

        def filler_step(k=1):
            for _ in range(k):
                if urgent:
                    urgent.pop(0)()
                    continue
                if pending_mul:
                    h, dr = pending_mul.pop(0)
                    emit_norm_mul(h, dr)
                    continue
                if filler:
                    filler.pop(0)()

        for pair in range(H // 2):
            hA, hB = 2 * pair, 2 * pair + 1
            es_a = p_e.tile([128, NT, N], F8, name="esa", tag="es")
            es_b = p_e.tile([128, NT, N], F8, name="esb", tag="es")
            # pv_A starts at kt=3: by then the previous pair's pv_B chunks
            # (popped from `urgent` at kt 0-3, drain included) have fully
            # released their pv-pool buffers, keeping pool rotation in
            # emission order (allocating earlier would race the rotation).
            pv_sched = {3: [0, 1], 5: [2], 7: [3]}
            pvsA = None
            for kt in range(NT):
                emit_scores_kt(pair, kt, es_a, es_b)
                filler_step(1)
                for j in pv_sched.get(kt, []):
                    if j == 0:
                        pvsA = [p_pv.tile([D + 1, 512], F32, name=f"pva{qc}", tag="pv")
                                for qc in range(QC)]
                    emit_pv_j(hA, es_a, pvsA, j)
            pending_mul.append((hA, emit_pv_drain(hA, pvsA)))

            pvsB = []

            def pv_b_chunk(j, h=hB, es=es_b, pvs=pvsB):
                if j == 0:
                    pvs += [p_pv.tile([D + 1, 512], F32, name=f"pvb{qc}", tag="pv")
                            for qc in range(QC)]
                emit_pv_j(h, es, pvs, j)
                if j == NP - 1:
                    pending_mul.append((h, emit_pv_drain(h, pvs)))
            for j in range(NP):
                urgent.append(lambda j=j: pv_b_chunk(j))

            if pair == H // 2 - 1:
                while urgent:
                    urgent.pop(0)()
            if pair == H // 2 - 3:
                # heads 0-7 normalize during the next pair; queue their
                # projection chunk behind those muls
                filler += [(lambda t=t: emit_proj_a(t)) for t in range(NT)]

        # drain the tail: remaining muls (heads 8-11), proj A remainder, proj B
        while pending_mul or filler:
            filler_step(1)
        for t in range(NT):
            emit_proj_b(t)

        if dbg is not None:
            for i in range(CK):
                nc.sync.dma_start(out=dbg["dbg_qT"][i], in_=qT[i][:])
                nc.sync.dma_start(out=dbg["dbg_kT"][i], in_=kT[i][:])
                nc.sync.dma_start(out=dbg["dbg_ao"][i], in_=aout[i][:])
            for j in range(NP):
                nc.sync.dma_start(out=dbg["dbg_vp"][j], in_=vpk[j][:, :, :, 0:D + 1])
            nc.sync.dma_start(out=dbg["dbg_esa"], in_=es_a[:])
            nc.sync.dma_start(out=dbg["dbg_esb"], in_=es_b[:])


_CACHE = {}


def kernel(x, pad_mask=None, w_attn=None, b_attn=None, w_proj=None, b_proj=None,
           _trace=False, _tmpdir=None):
    from concourse.bass_utils import run_bass_kernel_spmd

    FP8 = ml_dtypes.float8_e4m3

    x = np.asarray(x, dtype=np.float32)
    w_attn = np.asarray(w_attn, dtype=np.float32)
    b_attn = np.asarray(b_attn, dtype=np.float32)
    w_proj = np.asarray(w_proj, dtype=np.float32)
    b_proj = np.asarray(b_proj, dtype=np.float32)

    # host-side packing (free): transpose x
    xT = np.ascontiguousarray(
        x.transpose(0, 2, 1).reshape(B, CK, 128, N).astype(np.float16))
    waq16 = np.ascontiguousarray(
        w_attn[:, :2 * C].reshape(CK, 128, 2 * C)).astype(np.float16)
    wav16 = np.ascontiguousarray(
        w_attn[:, 2 * C:].reshape(CK, 128, C)).astype(np.float16)
    wp16 = np.ascontiguousarray(w_proj.reshape(CK, 128, C)).astype(np.float16)
    bq = np.ascontiguousarray(b_attn[:C].reshape(CK, 128).T).astype(np.float32)
    bp_eff = (b_proj + b_attn[2 * C:] @ w_proj).reshape(1, C).astype(np.float16)

    if "nc" not in _CACHE:
        _CACHE["nc"] = build_bass()
    nc = _CACHE["nc"]

    shared = {"waq": waq16, "wav": wav16, "wp": wp16, "bq": bq, "bp": bp_eff}
    in_maps = [dict(shared, xT=xT[b]) for b in range(B)]
    if _trace:
        _install_ntff_hook()
    res = run_bass_kernel_spmd(
        nc, in_maps, list(range(B)), trace=_trace, tmpdir=_tmpdir
    )
    out = np.stack([res.results[b]["y"].astype(np.float32) for b in range(B)],
                   axis=0)
    if _trace:
        return out, res
    return out
